# revision 23
# baseline (speedup 1.0000x reference)
import numpy as np

import concourse.bass as bass
import concourse.mybir as mybir
from concourse.tile import TileContext
from concourse.bass_utils import run_bass_kernel_spmd

F32 = mybir.dt.float32
BF16 = mybir.dt.bfloat16

B = 4
N = 2048
D = 1024
NQ = 1024
KH = 1024
NK = 2048
DV = 1024
NS = 8
P = 128
C = 512
SCALE = 1.0 / 32.0
EXT = [2 * (s + 1) for s in range(NS)]
EXTC = [e * P for e in EXT]
MW = 256
GROUPS = [[0, 1], [2, 3], [4, 5], [6, 7]]
BLOCKS = [[0, 2, 4, 6, 9, 11, 13, 15], [1, 3, 5, 7, 8, 10, 12, 14]]
DEPTH = 4
SWDGE = True


def _split_multi_waits(nc):
    eng = {
        mybir.EngineType.PE: "tensor",
        mybir.EngineType.Activation: "scalar",
        mybir.EngineType.DVE: "vector",
        mybir.EngineType.Pool: "gpsimd",
        mybir.EngineType.SP: "sync",
    }
    blocks = list(nc.m.functions[0].blocks)
    snapshots = [(b, list(b.instructions)) for b in blocks]
    new_lists = []
    for b, insts in snapshots:
        new_list = []
        for inst in insts:
            si = inst.sync_info
            waits = list(si.on_wait) if si and si.on_wait else []
            if len(waits) > 1:
                si.on_wait = waits[-1:]
                for w in waits[:-1]:
                    nop = getattr(nc, eng[inst.engine]).nop().ins
                    nsi = nop.sync_info
                    if nsi is None:
                        nop.sync_info = mybir.SyncInfo(on_wait=[w], on_update=[])
                    else:
                        nsi.on_wait = [w]
                        nsi.on_update = []
                    new_list.append(nop)
            new_list.append(inst)
        new_lists.append((b, new_list))
    for b, new_list in new_lists:
        b.instructions = new_list


def _build(repeat=1, surrogate=None):
    if surrogate is None:
        surrogate = repeat > 1
    nc = bass.Bass("TRN2", target_bir_lowering=False, debug=False, num_devices=8)

    xq_d = nc.dram_tensor("xq", [P, 8 * NQ], BF16, kind="ExternalInput").ap()
    xkv_d = nc.dram_tensor("xkv", [P, 8 * KH], BF16, kind="ExternalInput").ap()
    wq_d = nc.dram_tensor("wq", [P, 8 * 1024], BF16, kind="ExternalInput").ap()
    wk_d = nc.dram_tensor("wk", [P, 8 * 1024], BF16, kind="ExternalInput").ap()
    wv_d = nc.dram_tensor("wv", [P, 8 * 1024], BF16, kind="ExternalInput").ap()
    mask_d = nc.dram_tensor("masksb", [P, NS * MW], BF16, kind="ExternalInput").ap()
    id_d = nc.dram_tensor("ident", [P, P], BF16, kind="ExternalInput").ap()
    y_d = nc.dram_tensor("y", [NS, P, DV], BF16, kind="ExternalOutput").ap()
    agk_i = nc.dram_tensor("agk_i", [2, P, 8, C], BF16).ap()
    agk_o = nc.dram_tensor("agk_o", [2, 2, P, 8, C], BF16).ap()
    agv_i = nc.dram_tensor("agv_i", [2, P, 4, DV], BF16).ap()
    agv_o = nc.dram_tensor("agv_o", [2, 2, P, 4, DV], BF16).ap()

    args = (nc, xq_d, xkv_d, wq_d, wk_d, wv_d, mask_d, id_d, y_d,
            agk_i, agk_o, agv_i, agv_o, surrogate)
    with TileContext(nc, pool_alloc_mode="queue") as tc:
        if repeat == 1:
            _emit(tc, *args)
        else:
            with tc.For_i(0, repeat):
                _emit(tc, *args)

    _split_multi_waits(nc)
    return nc


def _cc(nc, surrogate, in_ap, out_ap, eng=None):
    if surrogate:
        flat_i = in_ap.rearrange("p a b -> p (a b)")
        for m in range(2):
            (eng or nc.sync).dma_start(
                out=out_ap[m].rearrange("p a b -> p (a b)"), in_=flat_i)
    else:
        nc.gpsimd.collective_compute(
            "AllGather", mybir.AluOpType.bypass, replica_groups=GROUPS,
            ins=[in_ap.opt()], outs=[out_ap.opt()])


def _emit(tc, nc, xq_d, xkv_d, wq_d, wk_d, wv_d, mask_d, id_d, y_d,
          agk_i, agk_o, agv_i, agv_o, surrogate=False):
    with tc.tile_pool(name="warm", bufs=1) as wmp, \
         tc.tile_pool(name="wmps", bufs=1, space="PSUM") as wmpp:
        wt = wmp.tile([P, P], BF16, tag="wm", name="wm")
        nc.gpsimd.memset(wt[:], 0.0)
        wps = wmpp.tile([P, P], F32, tag="wmp", name="wmp")
        NW = 32
        for i in range(NW):
            nc.tensor.matmul(wps[:], wt[:], wt[:],
                             start=(i == 0), stop=(i == NW - 1))
    with tc.tile_pool(name="qkv", bufs=1) as qkv:
        QT = qkv.tile([P, 8 * NQ], BF16, tag="qt", name="qt")
        KTq = [qkv.tile([P, 8 * C], BF16, tag=f"kt{q}", name=f"kt{q}")
               for q in range(4)]
        VT = qkv.tile([P, 16 * 1024], BF16, tag="vt", name="vt")
        maskt = qkv.tile([P, NS * MW], BF16, tag="mk", name="mk")
        ident = qkv.tile([P, P], BF16, tag="ident", name="ident")

        with tc.tile_pool(name="w", bufs=1) as wp:
            wqt = wp.tile([P, 8192], BF16, tag="wa", name="wqt")
            wkt = wp.tile([P, 8192], BF16, tag="wb", name="wkt")

            ppool = tc.tile_pool(name="pps", bufs=3, space="PSUM")
            pps = ppool.__enter__()
            with tc.tile_pool(name="xq", bufs=1) as xqp:
                xqt = xqp.tile([P, 8192], BF16, tag="xq", name="xqt")
                wq_v = wq_d.rearrange("p (d c) -> p d c", d=8)
                xq_v = xq_d.rearrange("p (d c) -> p d c", d=8)
                wqt_v = wqt[:].rearrange("p (d c) -> p d c", d=8)
                xqt_v = xqt[:].rearrange("p (d c) -> p d c", d=8)
                nc.scalar.dma_start(out=wqt_v[:, :, 0:256], in_=wq_v[:, :, 0:256])
                nc.sync.dma_start(out=xqt_v[:, :, 0:256], in_=xq_v[:, :, 0:256])
                nc.scalar.dma_start(out=wqt_v[:, :, 256:C], in_=wq_v[:, :, 256:C])
                nc.sync.dma_start(out=xqt_v[:, :, 256:C], in_=xq_v[:, :, 256:C])
                nc.scalar.dma_start(out=wqt_v[:, :, C:], in_=wq_v[:, :, C:])
                nc.sync.dma_start(out=xqt_v[:, :, C:], in_=xq_v[:, :, C:])
                nc.scalar.dma_start(out=ident[:], in_=id_d[:])
                nc.scalar.dma_start(out=wkt[:], in_=wk_d[:])
                nc.sync.dma_start(out=maskt[:], in_=mask_d[:])

                def q_group(qc, dk, w=C):
                    for o in range(0, C, w):
                        ps = pps.tile([P, C], F32, tag="pps",
                                      name=f"psq{dk}_{qc}_{o}")
                        c0 = qc * C + o
                        for d in range(8):
                            nc.tensor.matmul(
                                ps[:, :w],
                                wqt[:, d * 1024 + dk * P:d * 1024 + dk * P + P],
                                xqt[:, d * 1024 + c0:d * 1024 + c0 + w],
                                start=(d == 0), stop=(d == 7))
                        nc.scalar.mul(QT[:, dk * NQ + c0:dk * NQ + c0 + w],
                                      ps[:, :w], SCALE)

                q_group(0, 0, w=256)
                q_group(0, 1, w=256)
                for dk in range(2, 8):
                    q_group(0, dk)
                for dk in range(8):
                    q_group(1, dk)

            wvt = wp.tile([P, 8192], BF16, tag="wa", name="wvt")
            nc.scalar.dma_start(out=wvt[:], in_=wv_d[:])
            with tc.tile_pool(name="xkv", bufs=1) as xkp, \
                 tc.tile_pool(name="stg", bufs=2) as stp:
                xkt = xkp.tile([P, 8192], BF16, tag="xk", name="xkt")
                nc.sync.dma_start(out=xkt[:], in_=xkv_d[:])
                for sc in range(2):
                    kst = stp.tile([P, 8 * C], BF16, tag="kst", name=f"kst{sc}")
                    for dk in range(8):
                        ps = pps.tile([P, C], F32, tag="pps",
                                      name=f"psk{dk}_{sc}")
                        for d in range(8):
                            nc.tensor.matmul(
                                ps[:],
                                wkt[:, d * 1024 + dk * P:d * 1024 + dk * P + P],
                                xkt[:, d * 1024 + sc * C:d * 1024 + sc * C + C],
                                start=(d == 0), stop=(d == 7))
                        nc.vector.tensor_copy(kst[:, dk * C:(dk + 1) * C], ps[:])
                    nc.sync.dma_start(out=agk_i[sc], in_=kst[:])
                    _cc(nc, surrogate, agk_i[sc], agk_o[sc])
                    nc.sync.dma_start(
                        out=KTq[sc][:],
                        in_=agk_o[sc, 0].rearrange("p a b -> p (a b)"))
                    (nc.sync if (surrogate or not SWDGE) else nc.gpsimd).dma_start(
                        out=KTq[2 + sc][:],
                        in_=agk_o[sc, 1].rearrange("p a b -> p (a b)"))
                for vc in range(2):
                    vst = stp.tile([P, 4 * DV], BF16, tag="vst", name=f"vst{vc}")
                    for sub in range(4):
                        st = 4 * vc + sub
                        for vcc in range(2):
                            ps = pps.tile([P, C], F32, tag="pps",
                                          name=f"psv{st}_{vcc}")
                            for d in range(8):
                                nc.tensor.matmul(
                                    ps[:],
                                    xkt[:, d * 1024 + st * P:d * 1024 + st * P + P],
                                    wvt[:, d * 1024 + vcc * C:d * 1024 + vcc * C + C],
                                    start=(d == 0), stop=(d == 7))
                            nc.scalar.copy(
                                vst[:, sub * DV + vcc * C:sub * DV + vcc * C + C],
                                ps[:])
                    veng = nc.sync if (surrogate or not SWDGE) else nc.gpsimd
                    veng.dma_start(out=agv_i[vc], in_=vst[:])
                    _cc(nc, surrogate, agv_i[vc], agv_o[vc], eng=veng)
                    for m in range(2):
                        g0 = m * 8 + 4 * vc
                        veng.dma_start(
                            out=VT[:, g0 * 1024:(g0 + 4) * 1024],
                            in_=agv_o[vc, m])
            ppool.__exit__(None, None, None)

        with tc.tile_pool(name="at", bufs=2) as at, \
             tc.tile_pool(name="pb", bufs=DEPTH) as pb, \
             tc.tile_pool(name="stat", bufs=2 * (DEPTH + 1)) as stat, \
             tc.tile_pool(name="pts", bufs=16) as ptp, \
             tc.tile_pool(name="tps", bufs=3, space="PSUM") as tps, \
             tc.tile_pool(name="yps", bufs=3, space="PSUM") as yps, \
             tc.tile_pool(name="sps", bufs=2, space="PSUM") as sps:

            state = {}

            def s_phase(s):
                extc = EXTC[s]
                s_sb = at.tile([P, NK], F32, tag="s_sb", name=f"s_sb{s}")
                off = 0
                while off < extc:
                    w = min(C, extc - off)
                    ps = sps.tile([P, C], F32, tag="sps", name=f"sps{s}_{off}")
                    kq = KTq[off // C]
                    for dk in range(8):
                        nc.tensor.matmul(
                            ps[:, :w],
                            QT[:, dk * NQ + s * P:dk * NQ + (s + 1) * P],
                            kq[:, dk * C:dk * C + w],
                            start=(dk == 0), stop=(dk == 7))
                    mlo = extc - MW
                    if off + w <= mlo:
                        nc.vector.tensor_copy(s_sb[:, off:off + w], ps[:, :w])
                    else:
                        pw = max(0, mlo - off)
                        if pw:
                            nc.vector.tensor_copy(s_sb[:, off:off + pw],
                                                  ps[:, :pw])
                        nc.vector.tensor_tensor(
                            out=s_sb[:, off + pw:off + w], in0=ps[:, pw:w],
                            in1=maskt[:, s * MW + (off + pw - mlo):
                                      s * MW + (off + w - mlo)],
                            op=mybir.AluOpType.add)
                    off += w
                p_sb = pb.tile([P, NK], BF16, tag="p_sb", name=f"p_sb{s}")
                den = stat.tile([P, 1], F32, tag="den", name=f"den{s}")
                nc.scalar.activation(p_sb[:, :extc], s_sb[:, :extc],
                                     mybir.ActivationFunctionType.Exp,
                                     bias=0.0, scale=1.0, accum_out=den[:])
                rec = stat.tile([P, 1], F32, tag="rec", name=f"rec{s}")
                nc.vector.reciprocal(rec[:], den[:])
                state[s] = (p_sb, rec)

            def t_phase(s):
                ext = EXT[s]
                p_sb, rec = state.pop(s)
                pts = []
                for pr in range(ext // 2):
                    tp = tps.tile([P, 2 * P], BF16, tag="tps", name=f"tp{s}_{pr}")
                    for h in range(2):
                        ss = slice((2 * pr + h) * P, (2 * pr + h + 1) * P)
                        nc.tensor.transpose(tp[:, h * P:(h + 1) * P],
                                            p_sb[:, ss], ident[:])
                    pt = ptp.tile([P, 2 * P], BF16, tag="pt", name=f"pt{s}_{pr}")
                    nc.vector.tensor_copy(pt[:], tp[:])
                    pts.append(pt)
                state[s] = (pts, rec)

            def mm_phase(s):
                ext = EXT[s]
                pts, rec = state.pop(s)
                if s < NS - 1:
                    yt = [yps.tile([P, C], F32, tag="yps", name=f"yp{s}_{vc}")
                          for vc in range(2)]
                    for st in range(ext):
                        lhs = pts[st // 2][:, (st % 2) * P:(st % 2 + 1) * P]
                        for vc in range(2):
                            nc.tensor.matmul(
                                yt[vc][:], lhs,
                                VT[:, st * 1024 + vc * C:st * 1024 + vc * C + C],
                                start=(st == 0), stop=(st == ext - 1))
                    y_sb = at.tile([P, DV], BF16, tag="y_sb", name=f"ysb{s}")
                    for vc in range(2):
                        nc.scalar.activation(y_sb[:, vc * C:(vc + 1) * C],
                                             yt[vc][:],
                                             mybir.ActivationFunctionType.Copy,
                                             bias=0.0, scale=rec[:])
                        nc.scalar.dma_start(out=y_d[s, :, vc * C:(vc + 1) * C],
                                            in_=y_sb[:, vc * C:(vc + 1) * C])
                else:
                    y_sb = at.tile([P, DV], BF16, tag="y_sb", name=f"ysb{s}")
                    for vc in range(2):
                        yt = yps.tile([P, C], F32, tag="yps", name=f"yp{s}_{vc}")
                        for st in range(ext):
                            lhs = pts[st // 2][:, (st % 2) * P:(st % 2 + 1) * P]
                            nc.tensor.matmul(
                                yt[:], lhs,
                                VT[:, st * 1024 + vc * C:st * 1024 + vc * C + C],
                                start=(st == 0), stop=(st == ext - 1))
                        if vc == 0:
                            nc.scalar.activation(y_sb[:, 0:C], yt[:],
                                                 mybir.ActivationFunctionType.Copy,
                                                 bias=0.0, scale=rec[:])
                            nc.scalar.dma_start(out=y_d[s, :, 0:C],
                                                in_=y_sb[:, 0:C])
                        else:
                            for h in range(2):
                                hs = slice(C + h * 256, C + (h + 1) * 256)
                                nc.scalar.activation(
                                    y_sb[:, hs], yt[:, h * 256:(h + 1) * 256],
                                    mybir.ActivationFunctionType.Copy,
                                    bias=0.0, scale=rec[:])
                                nc.sync.dma_start(out=y_d[s, :, hs],
                                                  in_=y_sb[:, hs])

            for s in range(DEPTH):
                s_phase(s)
            t_phase(0)
            for s in range(NS):
                if s + DEPTH < NS:
                    s_phase(s + DEPTH)
                if s + 1 < NS:
                    t_phase(s + 1)
                mm_phase(s)


def _host_inputs(x, Wq, Wk, Wv):
    import ml_dtypes

    def perm(a):
        a = np.asarray(a, np.float32)
        X = a.shape[1]
        return np.ascontiguousarray(
            a.reshape(8, P, X).transpose(1, 0, 2).reshape(P, 8 * X)
        ).astype(ml_dtypes.bfloat16)

    wqT = np.asarray(Wq, np.float32).T
    wkT = np.asarray(Wk, np.float32).T
    wvT = np.asarray(Wv, np.float32).T
    wq_h, wk_h, wv_h = perm(wqT), perm(wkT), perm(wvT)
    ident = np.eye(P, dtype=ml_dtypes.bfloat16)
    row = np.arange(P)[:, None]
    mask_p = []
    for p in range(2):
        m = np.empty((P, NS * MW), np.float32)
        for s, j in enumerate(BLOCKS[p]):
            col = EXTC[s] - MW + np.arange(MW)[None, :]
            q = j * P + row
            m[:, s * MW:(s + 1) * MW] = np.where(col <= q, 0.0, -1e9)
        mask_p.append(m.astype(ml_dtypes.bfloat16))
    ins = []
    for c in range(8):
        b, p = c // 2, c % 2
        xb = np.asarray(x[b], dtype=np.float32)
        qidx = np.concatenate([np.arange(j * P, (j + 1) * P) for j in BLOCKS[p]])
        ins.append({
            "xq": perm(xb[qidx].T),
            "xkv": perm(xb[p * KH:(p + 1) * KH].T),
            "wq": wq_h, "wk": wk_h, "wv": wv_h,
            "masksb": mask_p[p],
            "ident": ident,
        })
    return ins


_NC_CACHE = []


def kernel(x, Wq, Wk, Wv):
    if not _NC_CACHE:
        _NC_CACHE.append(_build())
    nc = _NC_CACHE[0]
    ins = _host_inputs(x, Wq, Wk, Wv)
    res = run_bass_kernel_spmd(nc, ins, list(range(8))).results
    y = np.empty((B, N, DV), np.float32)
    for c in range(8):
        b, p = c // 2, c % 2
        for s, j in enumerate(BLOCKS[p]):
            y[b, j * P:(j + 1) * P] = res[c]["y"][s].astype(np.float32)
    return y
